# revision 32
# baseline (speedup 1.0000x reference)
"""Trainium2 Bass kernel for nn_EquivariantUpdate (GNN message passing).

Strategy: sort edges by destination (row), shard across 8 NeuronCores at
window boundaries (disjoint per-core aggregates, no collective).

v4: windows of 127 destination nodes. Rows are window-local, so instead
of gathering h[row] per edge we precompute A_w = H_w @ W1a^T per window
(one matmul), place w1c in its 128th row, and expand to edges with a
host-staged one-hot whose 128th row carries edge_attr — one matmul per
tile yields W1a h_row + w1c ea. The other one-hot orientation performs
the segment-sum. Col features use dma_gather (A/B halves for int16
range), with at most one mixed A/B tile per window (two col matmuls).
PSUM is batched in 4-tile groups: one SILU per layer per group, one
wide W2 matmul, batched phi PSUM->SBUF copy, and a single broadcast
tensor_tensor for the cd*phi scaling.
"""

import os
import numpy as np
import ml_dtypes

import concourse.bacc as bacc
import concourse.mybir as mybir
import concourse.tile as tile
from concourse.bass_utils import run_bass_kernel_spmd
from concourse.library_config import mlp as mlp_lib

H = 128
NCORES = 8
WIN = 127                      # nodes per aggregation window
NORM = 100.0
N_NODES = 50000                # overwritten per-call from input shapes
N_EDGES = 400000
HALF = 25000
BF16 = ml_dtypes.bfloat16
FP8 = ml_dtypes.float8_e4m3fn
CDSCALE = 131072.0             # cd prescale so fp8 cdp avoids denormals
SC = 32                        # tiles per DMA/gather chunk
GRP = 4                        # tiles per PSUM group (512 edges)

LAST_RUN_INFO = {}             # test.py reads exec_time_ns from here

_MAXW = 1


def _patch_drain():
    import concourse.tile as tile_mod
    if getattr(tile_mod.TileContext, "_eu_drain_patched", False):
        return
    ScopedClock = tile_mod.ScopedClock

    def _drain_and_barrier(self, tick_clock, wait_clock):
        nc = self.nc
        drain_inst = nc.sync.drain()
        wait_clock.add_sem_waits(
            drain_inst.ins, ScopedClock({None: tick_clock.global_clock})
        )
        inst = drain_inst.ins
        if inst.sync_info is not None and len(inst.sync_info.on_wait) > _MAXW:
            waits = list(inst.sync_info.on_wait)
            inst.sync_info.on_wait = waits[:_MAXW]
            for k in range(_MAXW, len(waits), _MAXW):
                extra = nc.sync.drain()
                einst = extra.ins
                if einst.sync_info is None:
                    einst.sync_info = mybir.SyncInfo(
                        on_wait=waits[k : k + _MAXW], on_update=[]
                    )
                else:
                    einst.sync_info.on_wait = waits[k : k + _MAXW]
        nc.all_engine_barrier()
        popped = nc._tile_sem_poison_stack.pop()
        assert popped is self._sem_poison
        nc.clear_and_free_semaphores(list(self.sems.allocated().values()))
        nc.all_engine_barrier()

    tile_mod.TileContext._drain_and_barrier = _drain_and_barrier
    tile_mod.TileContext._eu_drain_patched = True


def _wrap_idx(a):
    """[n] int16 -> [128, n//16] wrapped in 16 partitions, replicated x8."""
    n = a.shape[0]
    w = a.reshape(n // 16, 16).T
    return np.ascontiguousarray(np.tile(w, (8, 1)))


def _build_schedule(row, col):
    """Host-side scheduling. Returns static meta + per-core data."""
    n_win_total = (N_NODES + WIN - 1) // WIN

    perm = np.argsort(row, kind="stable")
    row_s = row[perm]
    col_s = col[perm]
    gwin = row_s // WIN  # global window id per sorted edge, non-decreasing

    wcount = np.bincount(gwin, minlength=n_win_total)
    cum = np.cumsum(wcount)
    bounds = [0]
    for c in range(1, NCORES):
        target = N_EDGES * c / NCORES
        bounds.append(int(np.searchsorted(cum, target)) + 1)
    bounds.append(n_win_total)
    w0 = bounds[:-1]
    w1 = bounds[1:]
    n_win = max(b - a for a, b in zip(w0, w1))

    wstart = np.concatenate([[0], cum]).astype(np.int64)
    half_of = (col_s >= HALF)

    core_win_half = []  # [core][w] -> (idxA, idxB) arrays of sorted-edge idx
    for c in range(NCORES):
        wins = []
        for w in range(n_win):
            g = w0[c] + w
            if g < w1[c]:
                lo, hi = wstart[g], wstart[g + 1]
                sl = np.arange(lo, hi)
                m = half_of[lo:hi]
                wins.append((sl[~m], sl[m]))
            else:
                wins.append((np.empty(0, np.int64), np.empty(0, np.int64)))
        core_win_half.append(wins)

    nAmax = np.zeros(n_win, np.int64)
    nBmax = np.zeros(n_win, np.int64)
    for w in range(n_win):
        for c in range(NCORES):
            a, b = core_win_half[c][w]
            nAmax[w] = max(nAmax[w], len(a))
            nBmax[w] = max(nBmax[w], len(b))
    # A-region sizes 16-aligned so gather offsets stay 16-aligned
    nAmax = -(-nAmax // 16) * 16

    TW = -(-(nAmax + nBmax) // 128)          # tiles per window
    nBslot = TW * 128 - nAmax                # B-region slots per window
    win_first = np.concatenate([[0], np.cumsum(TW)])[:-1]
    NT = int(TW.sum())
    NS = NT * 128

    meta = dict(
        n_win=n_win, NT=NT, NS=NS,
        win_first=win_first, win_ntiles=TW,
        nAmax=nAmax, nBslot=nBslot, w0=w0, w1=w1,
        nA=int(nAmax.sum()), nB=int(nBslot.sum()),
    )
    return meta, perm, row_s, col_s, core_win_half


def _tile_info(meta):
    """Static per-tile info: (window, ka) with ka = #A-slots in tile."""
    n_win = meta["n_win"]
    TW = meta["win_ntiles"]
    nAmax = meta["nAmax"]
    info = []
    for w in range(n_win):
        for k in range(int(TW[w])):
            ka = int(min(max(nAmax[w] - k * 128, 0), 128))
            info.append((w, ka))
    return info


def _stage_core(c, meta, inputs, perm, row_s, col_s, core_win_half,
                h_bf16, shared):
    """Build the per-core input map (slot-ordered staging arrays)."""
    n_win, NT, NS = meta["n_win"], meta["NT"], meta["NS"]
    win_first = meta["win_first"]
    nAmax, nBslot = meta["nAmax"], meta["nBslot"]
    w0 = meta["w0"]
    nb = w0[c] * WIN
    rmax = n_win * WIN

    coord = inputs["coord"]
    coord_diff = inputs["coord_diff"]
    edge_attr = inputs["edge_attr"]
    edge_mask = inputs["edge_mask"]
    node_mask = inputs["node_mask"]
    ucm = inputs["update_coords_mask"]

    # slot -> sorted-edge index (or -1 for padding)
    slot_edge = np.full(NS, -1, np.int64)
    slot_win = np.zeros(NS, np.int64)
    colidxA = np.zeros(_pad16(meta["nA"]), np.int16)
    colidxB = np.zeros(_pad16(meta["nB"]), np.int16)
    aoff = 0
    boff = 0
    for w in range(n_win):
        s0 = win_first[w] * 128
        ea_, eb_ = core_win_half[c][w]
        na, nbl = int(nAmax[w]), int(nBslot[w])
        slot_win[s0 : s0 + (na + nbl)] = w
        slot_edge[s0 : s0 + len(ea_)] = ea_
        slot_edge[s0 + na : s0 + na + len(eb_)] = eb_
        colidxA[aoff : aoff + len(ea_)] = col_s[ea_]
        colidxB[boff : boff + len(eb_)] = (col_s[eb_] - HALF)
        aoff += na
        boff += nbl

    valid = slot_edge >= 0
    se = np.where(valid, slot_edge, 0)

    rowv = row_s[se]
    loc = np.where(valid, rowv - nb - slot_win * WIN, 0)

    # one-hot tiles, both orientations (bf16); ohT row 127 carries ea
    tt = np.arange(NS) // 128
    ee = np.arange(NS) % 128
    v = valid
    ohT = np.zeros((128, NT, 128), FP8)    # [j, t, e]
    ohT[loc[v], tt[v], ee[v]] = 1.0
    eav = np.where(valid, edge_attr[perm[se], 0], 0.0).astype(np.float32)
    ohT[127, tt, ee] = eav.astype(FP8)
    oh = np.zeros((128, NT, 128), FP8)     # [e, t, j]
    oh[ee[v], tt[v], loc[v]] = 1.0

    em = np.where(valid, edge_mask[perm[se], 0], 0.0).astype(np.float32)
    cd = np.where(valid[:, None],
                  coord_diff[perm[se]] * (em * (CDSCALE / NORM))[:, None],
                  0.0).astype(np.float32)

    # transposed window blocks of h rows: hrowT[f, w*127 + j]
    avail = min(rmax, N_NODES - nb)
    blk = np.zeros((rmax, H), np.float32)
    blk[:avail] = np.asarray(h_bf16[nb : nb + avail], np.float32)
    hrowT = np.ascontiguousarray(
        blk.reshape(n_win, WIN, H).transpose(2, 0, 1).reshape(H, n_win * WIN)
    ).astype(BF16)

    def swz(x, rep3=False, scale=1.0):
        d = x.shape[1] if x.ndim > 1 else 1
        flat = np.zeros((rmax, d), np.float32)
        flat[:avail] = x[nb : nb + avail].reshape(avail, d) * scale
        out = flat.reshape(n_win, WIN, d)
        if rep3 and d == 1:
            out = np.repeat(out, 3, axis=2)
        out = out.transpose(1, 0, 2).reshape(WIN, -1)
        pad = np.zeros((128, out.shape[1]), np.float32)
        pad[:WIN] = out
        return np.ascontiguousarray(pad)

    in_map = {
        "h_full": h_bf16,
        "hrowT": hrowT,
        "colidxA": _wrap_idx(colidxA),
        "colidxB": _wrap_idx(colidxB),
        "ohT": np.ascontiguousarray(ohT.reshape(128, NT * 128)),
        "oh": np.ascontiguousarray(oh.reshape(128, NT * 128)),
        "cd": np.ascontiguousarray(
            cd.reshape(NT, 128, 3).transpose(1, 0, 2).astype(BF16)),
        "coordw": swz(coord),
        "ucm3": swz(ucm, rep3=True, scale=1.0 / CDSCALE),
        "nm3": swz(node_mask, rep3=True),
    }
    in_map.update(shared)
    return in_map


def _actfn():
    if os.environ.get("EU_SIM_ACT"):
        return mybir.ActivationFunctionType.Sigmoid
    return mybir.ActivationFunctionType.Silu


def _build_program(meta):
    n_win, NT, NS = meta["n_win"], meta["NT"], meta["NS"]
    win_first, win_ntiles = meta["win_first"], meta["win_ntiles"]
    nA, nB = meta["nA"], meta["nB"]
    rmax = n_win * WIN
    tinfo = _tile_info(meta)

    _patch_drain()
    nc = bacc.Bacc("TRN2", num_swdge_queues=4)
    dt = mybir.dt
    qrr = [0]

    def nextq():
        qrr[0] = (qrr[0] + 1) % 4
        return qrr[0]

    def P(name, shape, dtype, out=False):
        return nc.declare_dram_parameter(name, shape, dtype, isOutput=out)

    h_full = P("h_full", [N_NODES, H], dt.bfloat16)
    hrowT_d = P("hrowT", [H, rmax], dt.bfloat16)
    colidxA_d = P("colidxA", [128, _pad16(nA) // 16], dt.int16)
    colidxB_d = P("colidxB", [128, _pad16(nB) // 16], dt.int16)
    ohT_d = P("ohT", [128, NT * 128], dt.float8e4)
    oh_d = P("oh", [128, NT * 128], dt.float8e4)
    cd_d = P("cd", [128, NT, 3], dt.bfloat16)
    coordw_d = P("coordw", [128, n_win * 3], dt.float32)
    ucm3_d = P("ucm3", [128, n_win * 3], dt.float32)
    nm3_d = P("nm3", [128, n_win * 3], dt.float32)
    w1aT_d = P("w1aT", [H, H], dt.bfloat16)
    w1bT_d = P("w1bT", [H, H], dt.bfloat16)
    w1crep_d = P("w1crep", [1, n_win * 128], dt.float8e4)
    b1_d = P("b1", [H, 1], dt.float32)
    w2T_d = P("w2T", [H, H], dt.bfloat16)
    b2_d = P("b2", [H, 1], dt.float32)
    w3_d = P("w3", [H, 1], dt.bfloat16)
    out_d = P("out", [128, n_win * 3], dt.float32, out=True)

    nc.gpsimd.load_library(mlp_lib)

    # per-chunk static col-slot offsets (tapered: small first/last chunks)
    sizes = []
    t = 0
    lead = [8, 16, 24]
    while t < NT:
        rem = NT - t
        if lead:
            s = lead.pop(0)
        elif rem > SC + 24:
            s = SC
        elif rem > 24:
            s = max(8, rem // 2)
        else:
            s = rem
        s = min(s, rem)
        sizes.append(s)
        t += s
    chunk_t0 = []
    t = 0
    for s in sizes:
        chunk_t0.append(t)
        t += s
    a_off = [0]
    b_off = [0]
    for ci, t0 in enumerate(chunk_t0):
        t1c = min(t0 + sizes[ci], NT)
        ca = sum(tinfo[t][1] for t in range(t0, t1c))
        cb = sum(128 - tinfo[t][1] for t in range(t0, t1c))
        a_off.append(a_off[-1] + ca)
        b_off.append(b_off[-1] + cb)

    with tile.TileContext(nc) as tc:
        with (
            tc.tile_pool(name="const", bufs=1) as constp,
            tc.tile_pool(name="gath", bufs=6) as gathp,
            tc.tile_pool(name="work", bufs=3) as workp,
        ):
            # ---- constants ----
            w1aT = constp.tile([128, H], dt.bfloat16)
            nc.sync.dma_start(out=w1aT[:], in_=w1aT_d[:])
            w1bT = constp.tile([128, H], dt.bfloat16)
            nc.sync.dma_start(out=w1bT[:], in_=w1bT_d[:])
            b1 = constp.tile([H, 1], dt.float32)
            nc.sync.dma_start(out=b1[:], in_=b1_d[:])
            w2T = constp.tile([128, H], dt.bfloat16)
            nc.sync.dma_start(out=w2T[:], in_=w2T_d[:])
            b2 = constp.tile([H, 1], dt.float32)
            nc.sync.dma_start(out=b2[:], in_=b2_d[:])
            w3 = constp.tile([H, 1], dt.bfloat16)
            nc.sync.dma_start(out=w3[:], in_=w3_d[:])
            hrowT_sb = constp.tile([128, rmax], dt.bfloat16)
            nc.sync.dma_start(out=hrowT_sb[:], in_=hrowT_d[:])
            colA_sb = constp.tile([128, _pad16(nA) // 16], dt.int16)
            nc.scalar.dma_start(out=colA_sb[:], in_=colidxA_d[:])
            colB_sb = constp.tile([128, _pad16(nB) // 16], dt.int16)
            nc.scalar.dma_start(out=colB_sb[:], in_=colidxB_d[:])
            cd_sb = constp.tile([128, NT, 3], dt.bfloat16)
            nc.sync.dma_start(out=cd_sb[:], in_=cd_d[:])

            acc = constp.tile([128, n_win * 3], dt.float32)
            nc.vector.memset(acc[:], 0.0)

            awsb = constp.tile([128, n_win * 128], dt.float8e4)
            # row 127 of every window block = w1c (pairs with ea in ohT)
            nc.sync.dma_start(out=awsb[127:128, :], in_=w1crep_d[:])

            # ---- per-window A_w = H_w @ W1a^T (rows 0..126) ----
            with tc.tile_pool(name="awps", bufs=2, space="PSUM") as awpsp:
                for w in range(n_win):
                    a_ps = awpsp.tile([128, H], dt.float32, space="PSUM",
                                      tag="aps")
                    nc.tensor.matmul(a_ps[:WIN, :],
                                     hrowT_sb[:, w * WIN : (w + 1) * WIN],
                                     w1aT[:], start=True, stop=True)
                    nc.scalar.copy(awsb[:WIN, w * 128 : w * 128 + H],
                                   a_ps[:WIN, :])

            with (
                tc.tile_pool(name="mm1ps", bufs=2, space="PSUM") as mm1ps,
                tc.tile_pool(name="mm2ps", bufs=2, space="PSUM") as mm2ps,
                tc.tile_pool(name="phips", bufs=2, space="PSUM") as phips,
                tc.tile_pool(name="aggps", bufs=2, space="PSUM") as aggps,
            ):
                agg_state = [None]
                pending = []

                def emit_agg(p):
                    g0p, ngp, t0p, ohgp, cdpgp = p
                    for i in range(ngp):
                        t = g0p + i
                        w, ka = tinfo[t]
                        first = (t == win_first[w])
                        last = (t == win_first[w] + win_ntiles[w] - 1)
                        if first:
                            agg_state[0] = aggps.tile(
                                [128, 3], dt.float32, space="PSUM",
                                tag="agg", name="agg")
                        nc.tensor.matmul(
                            agg_state[0][:],
                            ohgp[:, (t - t0p) * 128 : (t - t0p + 1) * 128],
                            cdpgp[:, i, :], start=first, stop=last)
                        if last:
                            nc.vector.tensor_copy(
                                acc[:, w * 3 : (w + 1) * 3], agg_state[0][:])

                for ci, t0 in enumerate(chunk_t0):
                    t1 = min(t0 + sizes[ci], NT)
                    nrow = (t1 - t0) * 128
                    na_c = a_off[ci + 1] - a_off[ci]
                    nb_c = b_off[ci + 1] - b_off[ci]

                    bbase = -(-na_c // 128) * 128
                    cg = gathp.tile([128, 1, SC * 128 + 256], dt.bfloat16,
                                    tag="cg")
                    GC = 2048
                    for q0 in range(0, na_c, GC):
                        qn = -(-min(GC, na_c - q0) // 128) * 128
                        nc.gpsimd.dma_gather(
                            cg[:, :, q0 : q0 + qn], h_full[0:HALF],
                            colA_sb[:, (a_off[ci] + q0) // 16 :
                                    (a_off[ci] + q0 + qn) // 16],
                            qn, qn, H, transpose=True, single_packet=False,
                            queue_num=nextq())
                    for q0 in range(0, nb_c, GC):
                        qn = -(-min(GC, nb_c - q0) // 128) * 128
                        nc.gpsimd.dma_gather(
                            cg[:, :, bbase + q0 : bbase + q0 + qn],
                            h_full[HALF:N_NODES],
                            colB_sb[:, (b_off[ci] + q0) // 16 :
                                    (b_off[ci] + q0 + qn) // 16],
                            qn, qn, H, transpose=True, single_packet=False,
                            queue_num=nextq())

                    ohTg = gathp.tile([128, SC * 128], dt.float8e4, tag="ohTg")
                    nc.sync.dma_start(
                        out=ohTg[:, :nrow],
                        in_=ohT_d[:, t0 * 128 : t0 * 128 + nrow])
                    ohg = gathp.tile([128, SC * 128], dt.float8e4, tag="ohg")
                    nc.scalar.dma_start(
                        out=ohg[:, :nrow],
                        in_=oh_d[:, t0 * 128 : t0 * 128 + nrow])

                    apos = 0
                    bpos = 0
                    for g0 in range(t0, t1, GRP):
                        g1 = min(g0 + GRP, t1)
                        ng = g1 - g0
                        nge = ng * 128

                        ps1 = mm1ps.tile([128, GRP * 128], dt.float32,
                                         space="PSUM", tag="mm1")
                        for i in range(ng):
                            t = g0 + i
                            w, ka = tinfo[t]
                            sl = slice(i * 128, (i + 1) * 128)
                            nc.tensor.matmul(
                                ps1[:, sl],
                                awsb[:, w * 128 : (w + 1) * 128],
                                ohTg[:, (t - t0) * 128 : (t - t0 + 1) * 128],
                                start=True, stop=False)
                            if ka > 0:
                                nc.tensor.matmul(
                                    ps1[:, i * 128 : i * 128 + ka], w1bT[:],
                                    cg[:, 0, apos : apos + ka],
                                    start=False, stop=True)
                                apos += ka
                            if ka < 128:
                                kb = 128 - ka
                                nc.tensor.matmul(
                                    ps1[:, i * 128 + ka : (i + 1) * 128],
                                    w1bT[:],
                                    cg[:, 0, bbase + bpos : bbase + bpos + kb],
                                    start=False, stop=True)
                                bpos += kb

                        x1 = workp.tile([128, GRP * 128], dt.bfloat16,
                                        tag="x1")
                        nc.scalar.activation(x1[:, :nge], ps1[:, :nge],
                                             _actfn(), bias=b1[:])
                        ps2 = mm2ps.tile([128, GRP * 128], dt.float32,
                                         space="PSUM", tag="mm2")
                        nc.tensor.matmul(ps2[:, :nge], w2T[:], x1[:, :nge],
                                         start=True, stop=True)
                        if pending:
                            emit_agg(pending.pop(0))
                        x2 = workp.tile([128, GRP * 128], dt.bfloat16,
                                        tag="x2")
                        nc.scalar.activation(x2[:, :nge], ps2[:, :nge],
                                             _actfn(), bias=b2[:])

                        phi_ps = phips.tile([128, GRP], dt.float32,
                                            space="PSUM", tag="phi")
                        for i in range(ng):
                            nc.tensor.matmul(phi_ps[:, i : i + 1],
                                             x2[:, i * 128 : (i + 1) * 128],
                                             w3[:], start=True, stop=True)
                        phi_sb = workp.tile([128, GRP], dt.float32,
                                            tag="phisb")
                        nc.vector.tensor_copy(phi_sb[:, :ng], phi_ps[:, :ng])
                        cdpg = workp.tile([128, GRP, 3], dt.float8e4,
                                          tag="cdpg")
                        phib = phi_sb[:, :ng].unsqueeze(2).broadcast_to(
                            [128, ng, 3])
                        nc.vector.tensor_tensor(
                            cdpg[:, :ng, :], cd_sb[:, g0 : g0 + ng, :], phib,
                            op=mybir.AluOpType.mult)

                        pending.append((g0, ng, t0, ohg, cdpg))

                while pending:
                    emit_agg(pending.pop(0))

                # ---- final coord update ----
                coordw = constp.tile([128, n_win * 3], dt.float32)
                nc.sync.dma_start(out=coordw[:], in_=coordw_d[:])
                ucm3 = constp.tile([128, n_win * 3], dt.float32)
                nc.sync.dma_start(out=ucm3[:], in_=ucm3_d[:])
                nm3 = constp.tile([128, n_win * 3], dt.float32)
                nc.sync.dma_start(out=nm3[:], in_=nm3_d[:])
                outw = constp.tile([128, n_win * 3], dt.float32)
                nc.vector.tensor_tensor(acc[:], acc[:], ucm3[:],
                                        op=mybir.AluOpType.mult)
                nc.vector.tensor_tensor(outw[:], acc[:], coordw[:],
                                        op=mybir.AluOpType.add)
                nc.vector.tensor_tensor(outw[:], outw[:], nm3[:],
                                        op=mybir.AluOpType.mult)
                nc.sync.dma_start(out=out_d[:], in_=outw[:])

    nc.compile()
    return nc


def _pad16(n):
    # 16-aligned + 128 slack entries (last gather call rounds up to x128)
    return max(-(-n // 16) * 16, 128) + 128


def kernel(**inputs):
    global N_NODES, N_EDGES, HALF
    h = np.asarray(inputs["h"], np.float32)
    N_NODES = h.shape[0]
    N_EDGES = np.asarray(inputs["edge_index"]).shape[1]
    HALF = (N_NODES + 1) // 2
    assert HALF < 32768 and N_NODES - HALF < 32768
    coord = np.asarray(inputs["coord"], np.float32)
    edge_index = np.asarray(inputs["edge_index"]).astype(np.int64)
    row, col = edge_index[0], edge_index[1]

    ins = dict(inputs)
    ins["coord"] = coord

    meta, perm, row_s, col_s, cwh = _build_schedule(row, col)
    h_bf16 = np.ascontiguousarray(h.astype(BF16))

    W1 = np.asarray(inputs["W1"], np.float32)
    W2 = np.asarray(inputs["W2"], np.float32)
    W3 = np.asarray(inputs["W3"], np.float32)
    w1c = W1[:, 2 * H].astype(BF16)                      # [H]
    shared = {
        "w1aT": np.ascontiguousarray(W1[:, :H].T.astype(BF16)),
        "w1bT": np.ascontiguousarray(W1[:, H : 2 * H].T.astype(BF16)),
        "w1crep": np.ascontiguousarray(
            np.tile(w1c, meta["n_win"]).reshape(1, -1).astype(FP8)),
        "b1": np.asarray(inputs["b1"], np.float32).reshape(H, 1),
        "w2T": np.ascontiguousarray(W2.T.astype(BF16)),
        "b2": np.asarray(inputs["b2"], np.float32).reshape(H, 1),
        "w3": np.ascontiguousarray(W3.reshape(1, H).T.astype(BF16)),
    }

    in_maps = [
        _stage_core(c, meta, ins, perm, row_s, col_s, cwh, h_bf16, shared)
        for c in range(NCORES)
    ]

    nc = _build_program(meta)
    trace = bool(os.environ.get("EU_TRACE"))
    res = run_bass_kernel_spmd(nc, in_maps, list(range(NCORES)), trace=trace)
    LAST_RUN_INFO["exec_time_ns"] = res.exec_time_ns

    n_win = meta["n_win"]
    out = np.empty((N_NODES, 3), np.float32)
    for c in range(NCORES):
        nb = meta["w0"][c] * WIN
        ne = min(meta["w1"][c] * WIN, N_NODES)
        arr = res.results[c]["out"].reshape(128, n_win, 3)[:WIN]
        arr = np.ascontiguousarray(arr.transpose(1, 0, 2)).reshape(-1, 3)
        out[nb:ne] = arr[: ne - nb]
    return out


# revision 33
# speedup vs baseline: 1.2073x; 1.2073x over previous
"""Trainium2 Bass kernel for nn_EquivariantUpdate (GNN message passing).

Strategy: sort edges by destination (row), shard across 8 NeuronCores at
window boundaries (disjoint per-core aggregates, no collective).

v4: windows of 127 destination nodes. Rows are window-local, so instead
of gathering h[row] per edge we precompute A_w = H_w @ W1a^T per window
(one matmul), place w1c in its 128th row, and expand to edges with a
host-staged one-hot whose 128th row carries edge_attr — one matmul per
tile yields W1a h_row + w1c ea. The other one-hot orientation performs
the segment-sum. Col features use dma_gather (A/B halves for int16
range), with at most one mixed A/B tile per window (two col matmuls).
PSUM is batched in 4-tile groups: one SILU per layer per group, one
wide W2 matmul, batched phi PSUM->SBUF copy, and a single broadcast
tensor_tensor for the cd*phi scaling.
"""

import os
import numpy as np
import ml_dtypes

import concourse.bacc as bacc
import concourse.mybir as mybir
import concourse.tile as tile
from concourse.bass_utils import run_bass_kernel_spmd
from concourse.library_config import mlp as mlp_lib

H = 128
NCORES = 8
WIN = 127                      # nodes per aggregation window
NORM = 100.0
N_NODES = 50000                # overwritten per-call from input shapes
N_EDGES = 400000
HALF = 25000
BF16 = ml_dtypes.bfloat16
FP8 = ml_dtypes.float8_e4m3fn
CDSCALE = 131072.0             # cd prescale so fp8 cdp avoids denormals
SC = 32                        # tiles per DMA/gather chunk
GRP = 4                        # tiles per PSUM group (512 edges)

LAST_RUN_INFO = {}             # test.py reads exec_time_ns from here

_MAXW = 1


def _patch_drain():
    import concourse.tile as tile_mod
    if getattr(tile_mod.TileContext, "_eu_drain_patched", False):
        return
    ScopedClock = tile_mod.ScopedClock

    def _drain_and_barrier(self, tick_clock, wait_clock):
        nc = self.nc
        drain_inst = nc.sync.drain()
        wait_clock.add_sem_waits(
            drain_inst.ins, ScopedClock({None: tick_clock.global_clock})
        )
        inst = drain_inst.ins
        if inst.sync_info is not None and len(inst.sync_info.on_wait) > _MAXW:
            waits = list(inst.sync_info.on_wait)
            inst.sync_info.on_wait = waits[:_MAXW]
            for k in range(_MAXW, len(waits), _MAXW):
                extra = nc.sync.drain()
                einst = extra.ins
                if einst.sync_info is None:
                    einst.sync_info = mybir.SyncInfo(
                        on_wait=waits[k : k + _MAXW], on_update=[]
                    )
                else:
                    einst.sync_info.on_wait = waits[k : k + _MAXW]
        nc.all_engine_barrier()
        popped = nc._tile_sem_poison_stack.pop()
        assert popped is self._sem_poison
        nc.clear_and_free_semaphores(list(self.sems.allocated().values()))
        nc.all_engine_barrier()

    tile_mod.TileContext._drain_and_barrier = _drain_and_barrier
    tile_mod.TileContext._eu_drain_patched = True


def _wrap_idx(a):
    """[n] int16 -> [128, n//16] wrapped in 16 partitions, replicated x8."""
    n = a.shape[0]
    w = a.reshape(n // 16, 16).T
    return np.ascontiguousarray(np.tile(w, (8, 1)))


def _build_schedule(row, col):
    """Host-side scheduling. Returns static meta + per-core data."""
    n_win_total = (N_NODES + WIN - 1) // WIN

    perm = np.argsort(row, kind="stable")
    row_s = row[perm]
    col_s = col[perm]
    gwin = row_s // WIN  # global window id per sorted edge, non-decreasing

    wcount = np.bincount(gwin, minlength=n_win_total)
    cum = np.cumsum(wcount)
    bounds = [0]
    for c in range(1, NCORES):
        target = N_EDGES * c / NCORES
        bounds.append(int(np.searchsorted(cum, target)) + 1)
    bounds.append(n_win_total)
    w0 = bounds[:-1]
    w1 = bounds[1:]
    n_win = max(b - a for a, b in zip(w0, w1))

    wstart = np.concatenate([[0], cum]).astype(np.int64)
    half_of = (col_s >= HALF)

    core_win_half = []  # [core][w] -> (idxA, idxB) arrays of sorted-edge idx
    for c in range(NCORES):
        wins = []
        for w in range(n_win):
            g = w0[c] + w
            if g < w1[c]:
                lo, hi = wstart[g], wstart[g + 1]
                sl = np.arange(lo, hi)
                m = half_of[lo:hi]
                wins.append((sl[~m], sl[m]))
            else:
                wins.append((np.empty(0, np.int64), np.empty(0, np.int64)))
        core_win_half.append(wins)

    nAmax = np.zeros(n_win, np.int64)
    nBmax = np.zeros(n_win, np.int64)
    for w in range(n_win):
        for c in range(NCORES):
            a, b = core_win_half[c][w]
            nAmax[w] = max(nAmax[w], len(a))
            nBmax[w] = max(nBmax[w], len(b))
    # A-region sizes 16-aligned so gather offsets stay 16-aligned
    nAmax = -(-nAmax // 16) * 16

    TW = -(-(nAmax + nBmax) // 128)          # tiles per window
    nBslot = TW * 128 - nAmax                # B-region slots per window
    win_first = np.concatenate([[0], np.cumsum(TW)])[:-1]
    NT = int(TW.sum())
    NS = NT * 128

    meta = dict(
        n_win=n_win, NT=NT, NS=NS,
        win_first=win_first, win_ntiles=TW,
        nAmax=nAmax, nBslot=nBslot, w0=w0, w1=w1,
        nA=int(nAmax.sum()), nB=int(nBslot.sum()),
    )
    return meta, perm, row_s, col_s, core_win_half


def _tile_info(meta):
    """Static per-tile info: (window, ka) with ka = #A-slots in tile."""
    n_win = meta["n_win"]
    TW = meta["win_ntiles"]
    nAmax = meta["nAmax"]
    info = []
    for w in range(n_win):
        for k in range(int(TW[w])):
            ka = int(min(max(nAmax[w] - k * 128, 0), 128))
            info.append((w, ka))
    return info


def _stage_core(c, meta, inputs, perm, row_s, col_s, core_win_half,
                h_bf16, shared):
    """Build the per-core input map (slot-ordered staging arrays)."""
    n_win, NT, NS = meta["n_win"], meta["NT"], meta["NS"]
    win_first = meta["win_first"]
    nAmax, nBslot = meta["nAmax"], meta["nBslot"]
    w0 = meta["w0"]
    nb = w0[c] * WIN
    rmax = n_win * WIN

    coord = inputs["coord"]
    coord_diff = inputs["coord_diff"]
    edge_attr = inputs["edge_attr"]
    edge_mask = inputs["edge_mask"]
    node_mask = inputs["node_mask"]
    ucm = inputs["update_coords_mask"]

    # slot -> sorted-edge index (or -1 for padding)
    slot_edge = np.full(NS, -1, np.int64)
    slot_win = np.zeros(NS, np.int64)
    colidxA = np.zeros(_pad16(meta["nA"]), np.int16)
    colidxB = np.zeros(_pad16(meta["nB"]), np.int16)
    aoff = 0
    boff = 0
    for w in range(n_win):
        s0 = win_first[w] * 128
        ea_, eb_ = core_win_half[c][w]
        na, nbl = int(nAmax[w]), int(nBslot[w])
        slot_win[s0 : s0 + (na + nbl)] = w
        slot_edge[s0 : s0 + len(ea_)] = ea_
        slot_edge[s0 + na : s0 + na + len(eb_)] = eb_
        colidxA[aoff : aoff + len(ea_)] = col_s[ea_]
        colidxB[boff : boff + len(eb_)] = (col_s[eb_] - HALF)
        aoff += na
        boff += nbl

    valid = slot_edge >= 0
    se = np.where(valid, slot_edge, 0)

    rowv = row_s[se]
    loc = np.where(valid, rowv - nb - slot_win * WIN, 0)

    # one-hot tiles, both orientations (bf16); ohT row 127 carries ea
    tt = np.arange(NS) // 128
    ee = np.arange(NS) % 128
    v = valid
    ohT = np.zeros((128, NT, 128), FP8)    # [j, t, e]
    ohT[loc[v], tt[v], ee[v]] = 1.0
    eav = np.where(valid, edge_attr[perm[se], 0], 0.0).astype(np.float32)
    ohT[127, tt, ee] = eav.astype(FP8)
    oh = np.zeros((128, NT, 128), FP8)     # [e, t, j]
    oh[ee[v], tt[v], loc[v]] = 1.0

    em = np.where(valid, edge_mask[perm[se], 0], 0.0).astype(np.float32)
    cd = np.where(valid[:, None],
                  coord_diff[perm[se]] * (em * (CDSCALE / NORM))[:, None],
                  0.0).astype(np.float32)

    # transposed window blocks of h rows: hrowT[f, w*127 + j]
    avail = min(rmax, N_NODES - nb)
    blk = np.zeros((rmax, H), np.float32)
    blk[:avail] = np.asarray(h_bf16[nb : nb + avail], np.float32)
    hrowT = np.ascontiguousarray(
        blk.reshape(n_win, WIN, H).transpose(2, 0, 1).reshape(H, n_win * WIN)
    ).astype(BF16)

    def swz(x, rep3=False, scale=1.0):
        d = x.shape[1] if x.ndim > 1 else 1
        flat = np.zeros((rmax, d), np.float32)
        flat[:avail] = x[nb : nb + avail].reshape(avail, d) * scale
        out = flat.reshape(n_win, WIN, d)
        if rep3 and d == 1:
            out = np.repeat(out, 3, axis=2)
        out = out.transpose(1, 0, 2).reshape(WIN, -1)
        pad = np.zeros((128, out.shape[1]), np.float32)
        pad[:WIN] = out
        return np.ascontiguousarray(pad)

    in_map = {
        "h_full": h_bf16,
        "hrowT": hrowT,
        "colidxA": _wrap_idx(colidxA),
        "colidxB": _wrap_idx(colidxB),
        "ohT": np.ascontiguousarray(ohT.reshape(128, NT * 128)),
        "oh": np.ascontiguousarray(oh.reshape(128, NT * 128)),
        "cd": np.ascontiguousarray(
            cd.reshape(NT, 128, 3).transpose(1, 0, 2).astype(BF16)),
        "coordw": swz(coord),
        "ucm3": swz(ucm, rep3=True, scale=1.0 / CDSCALE),
        "nm3": swz(node_mask, rep3=True),
    }
    in_map.update(shared)
    return in_map


def _actfn():
    if os.environ.get("EU_SIM_ACT"):
        return mybir.ActivationFunctionType.Sigmoid
    return mybir.ActivationFunctionType.Silu


def _build_program(meta):
    n_win, NT, NS = meta["n_win"], meta["NT"], meta["NS"]
    win_first, win_ntiles = meta["win_first"], meta["win_ntiles"]
    nA, nB = meta["nA"], meta["nB"]
    rmax = n_win * WIN
    tinfo = _tile_info(meta)

    _patch_drain()
    nc = bacc.Bacc("TRN2", num_swdge_queues=4)
    dt = mybir.dt
    qrr = [0]

    def nextq():
        qrr[0] = (qrr[0] + 1) % 4
        return qrr[0]

    def P(name, shape, dtype, out=False):
        return nc.declare_dram_parameter(name, shape, dtype, isOutput=out)

    h_full = P("h_full", [N_NODES, H], dt.bfloat16)
    hrowT_d = P("hrowT", [H, rmax], dt.bfloat16)
    colidxA_d = P("colidxA", [128, _pad16(nA) // 16], dt.int16)
    colidxB_d = P("colidxB", [128, _pad16(nB) // 16], dt.int16)
    ohT_d = P("ohT", [128, NT * 128], dt.float8e4)
    oh_d = P("oh", [128, NT * 128], dt.float8e4)
    cd_d = P("cd", [128, NT, 3], dt.bfloat16)
    coordw_d = P("coordw", [128, n_win * 3], dt.float32)
    ucm3_d = P("ucm3", [128, n_win * 3], dt.float32)
    nm3_d = P("nm3", [128, n_win * 3], dt.float32)
    w1aT_d = P("w1aT", [H, H], dt.bfloat16)
    w1bT_d = P("w1bT", [H, H], dt.bfloat16)
    w1crep_d = P("w1crep", [1, n_win * 128], dt.float8e4)
    b1_d = P("b1", [H, 1], dt.float32)
    w2T_d = P("w2T", [H, H], dt.bfloat16)
    b2_d = P("b2", [H, 1], dt.float32)
    w3_d = P("w3", [H, 1], dt.bfloat16)
    out_d = P("out", [128, n_win * 3], dt.float32, out=True)

    nc.gpsimd.load_library(mlp_lib)

    # per-chunk static col-slot offsets (tapered: small first/last chunks)
    sizes = []
    t = 0
    lead = [8, 16, 24]
    while t < NT:
        rem = NT - t
        if lead:
            s = lead.pop(0)
        elif rem > SC + 24:
            s = SC
        elif rem > 24:
            s = max(8, rem // 2)
        else:
            s = rem
        s = min(s, rem)
        sizes.append(s)
        t += s
    chunk_t0 = []
    t = 0
    for s in sizes:
        chunk_t0.append(t)
        t += s
    a_off = [0]
    b_off = [0]
    for ci, t0 in enumerate(chunk_t0):
        t1c = min(t0 + sizes[ci], NT)
        ca = sum(tinfo[t][1] for t in range(t0, t1c))
        cb = sum(128 - tinfo[t][1] for t in range(t0, t1c))
        a_off.append(a_off[-1] + ca)
        b_off.append(b_off[-1] + cb)

    with tile.TileContext(nc) as tc:
        with (
            tc.tile_pool(name="const", bufs=1) as constp,
            tc.tile_pool(name="gath", bufs=6) as gathp,
            tc.tile_pool(name="work", bufs=3) as workp,
        ):
            # ---- constants ----
            w1aT = constp.tile([128, H], dt.bfloat16)
            nc.sync.dma_start(out=w1aT[:], in_=w1aT_d[:])
            w1bT = constp.tile([128, H], dt.bfloat16)
            nc.sync.dma_start(out=w1bT[:], in_=w1bT_d[:])
            b1 = constp.tile([H, 1], dt.float32)
            nc.sync.dma_start(out=b1[:], in_=b1_d[:])
            w2T = constp.tile([128, H], dt.bfloat16)
            nc.sync.dma_start(out=w2T[:], in_=w2T_d[:])
            b2 = constp.tile([H, 1], dt.float32)
            nc.sync.dma_start(out=b2[:], in_=b2_d[:])
            w3 = constp.tile([H, 1], dt.bfloat16)
            nc.sync.dma_start(out=w3[:], in_=w3_d[:])
            hrowT_sb = constp.tile([128, rmax], dt.bfloat16)
            nc.sync.dma_start(out=hrowT_sb[:], in_=hrowT_d[:])
            colA_sb = constp.tile([128, _pad16(nA) // 16], dt.int16)
            nc.scalar.dma_start(out=colA_sb[:], in_=colidxA_d[:])
            colB_sb = constp.tile([128, _pad16(nB) // 16], dt.int16)
            nc.scalar.dma_start(out=colB_sb[:], in_=colidxB_d[:])
            cd_sb = constp.tile([128, NT, 3], dt.bfloat16)
            nc.sync.dma_start(out=cd_sb[:], in_=cd_d[:])

            acc = constp.tile([128, n_win * 3], dt.float32)
            nc.vector.memset(acc[:], 0.0)

            awsb = constp.tile([128, n_win * 128], dt.float8e4)
            # row 127 of every window block = w1c (pairs with ea in ohT)
            nc.sync.dma_start(out=awsb[127:128, :], in_=w1crep_d[:])

            # ---- per-window A_w = H_w @ W1a^T (rows 0..126) ----
            with tc.tile_pool(name="awps", bufs=2, space="PSUM") as awpsp:
                for w in range(n_win):
                    a_ps = awpsp.tile([128, H], dt.float32, space="PSUM",
                                      tag="aps")
                    nc.tensor.matmul(a_ps[:WIN, :],
                                     hrowT_sb[:, w * WIN : (w + 1) * WIN],
                                     w1aT[:], start=True, stop=True)
                    nc.scalar.copy(awsb[:WIN, w * 128 : w * 128 + H],
                                   a_ps[:WIN, :])

            with (
                tc.tile_pool(name="mm1ps", bufs=2, space="PSUM") as mm1ps,
                tc.tile_pool(name="mm2ps", bufs=2, space="PSUM") as mm2ps,
                tc.tile_pool(name="phips", bufs=2, space="PSUM") as phips,
                tc.tile_pool(name="aggps", bufs=2, space="PSUM") as aggps,
            ):
                agg_state = [None]
                pending = []

                def emit_agg(p):
                    g0p, ngp, t0p, ohgp, cdpgp = p
                    for i in range(ngp):
                        t = g0p + i
                        w, ka = tinfo[t]
                        first = (t == win_first[w])
                        last = (t == win_first[w] + win_ntiles[w] - 1)
                        if first:
                            agg_state[0] = aggps.tile(
                                [128, 3], dt.float32, space="PSUM",
                                tag="agg", name="agg")
                        nc.tensor.matmul(
                            agg_state[0][:],
                            ohgp[:, (t - t0p) * 128 : (t - t0p + 1) * 128],
                            cdpgp[:, i, :], start=first, stop=last)
                        if last:
                            nc.vector.tensor_copy(
                                acc[:, w * 3 : (w + 1) * 3], agg_state[0][:])

                for ci, t0 in enumerate(chunk_t0):
                    t1 = min(t0 + sizes[ci], NT)
                    nrow = (t1 - t0) * 128
                    na_c = a_off[ci + 1] - a_off[ci]
                    nb_c = b_off[ci + 1] - b_off[ci]

                    bbase = -(-na_c // 128) * 128
                    cg = gathp.tile([128, 1, SC * 128 + 256], dt.bfloat16,
                                    tag="cg")
                    GC = 2048
                    for q0 in range(0, na_c, GC):
                        qn = -(-min(GC, na_c - q0) // 128) * 128
                        nc.gpsimd.dma_gather(
                            cg[:, :, q0 : q0 + qn], h_full[0:HALF],
                            colA_sb[:, (a_off[ci] + q0) // 16 :
                                    (a_off[ci] + q0 + qn) // 16],
                            qn, qn, H, transpose=True, single_packet=False,
                            queue_num=nextq())
                    for q0 in range(0, nb_c, GC):
                        qn = -(-min(GC, nb_c - q0) // 128) * 128
                        nc.gpsimd.dma_gather(
                            cg[:, :, bbase + q0 : bbase + q0 + qn],
                            h_full[HALF:N_NODES],
                            colB_sb[:, (b_off[ci] + q0) // 16 :
                                    (b_off[ci] + q0 + qn) // 16],
                            qn, qn, H, transpose=True, single_packet=False,
                            queue_num=nextq())

                    ohTg = gathp.tile([128, SC * 128], dt.float8e4, tag="ohTg")
                    nc.sync.dma_start(
                        out=ohTg[:, :nrow],
                        in_=ohT_d[:, t0 * 128 : t0 * 128 + nrow])
                    ohg = gathp.tile([128, SC * 128], dt.float8e4, tag="ohg")
                    nc.scalar.dma_start(
                        out=ohg[:, :nrow],
                        in_=oh_d[:, t0 * 128 : t0 * 128 + nrow])

                    apos = 0
                    bpos = 0
                    for g0 in range(t0, t1, GRP):
                        g1 = min(g0 + GRP, t1)
                        ng = g1 - g0
                        nge = ng * 128

                        ps1 = mm1ps.tile([128, GRP * 128], dt.float32,
                                         space="PSUM", tag="mm1")
                        for i in range(ng):
                            t = g0 + i
                            w, ka = tinfo[t]
                            sl = slice(i * 128, (i + 1) * 128)
                            nc.tensor.matmul(
                                ps1[:, sl],
                                awsb[:, w * 128 : (w + 1) * 128],
                                ohTg[:, (t - t0) * 128 : (t - t0 + 1) * 128],
                                start=True, stop=False)
                            if ka > 0:
                                nc.tensor.matmul(
                                    ps1[:, i * 128 : i * 128 + ka], w1bT[:],
                                    cg[:, 0, apos : apos + ka],
                                    start=False, stop=True)
                                apos += ka
                            if ka < 128:
                                kb = 128 - ka
                                nc.tensor.matmul(
                                    ps1[:, i * 128 + ka : (i + 1) * 128],
                                    w1bT[:],
                                    cg[:, 0, bbase + bpos : bbase + bpos + kb],
                                    start=False, stop=True)
                                bpos += kb

                        x1 = workp.tile([128, GRP * 128], dt.bfloat16,
                                        tag="x1")
                        nc.scalar.activation(x1[:, :nge], ps1[:, :nge],
                                             _actfn(), bias=b1[:])
                        ps2 = mm2ps.tile([128, GRP * 128], dt.float32,
                                         space="PSUM", tag="mm2")
                        nc.tensor.matmul(ps2[:, :nge], w2T[:], x1[:, :nge],
                                         start=True, stop=True)
                        x2 = workp.tile([128, GRP * 128], dt.bfloat16,
                                        tag="x2")
                        nc.scalar.activation(x2[:, :nge], ps2[:, :nge],
                                             _actfn(), bias=b2[:])

                        phi_ps = phips.tile([128, GRP], dt.float32,
                                            space="PSUM", tag="phi")
                        for i in range(ng):
                            nc.tensor.matmul(phi_ps[:, i : i + 1],
                                             x2[:, i * 128 : (i + 1) * 128],
                                             w3[:], start=True, stop=True)
                        phi_sb = workp.tile([128, GRP], dt.float32,
                                            tag="phisb")
                        nc.vector.tensor_copy(phi_sb[:, :ng], phi_ps[:, :ng])
                        cdpg = workp.tile([128, GRP, 3], dt.float8e4,
                                          tag="cdpg")
                        phib = phi_sb[:, :ng].unsqueeze(2).broadcast_to(
                            [128, ng, 3])
                        nc.vector.tensor_tensor(
                            cdpg[:, :ng, :], cd_sb[:, g0 : g0 + ng, :], phib,
                            op=mybir.AluOpType.mult)

                        pending.append((g0, ng, t0, ohg, cdpg))
                        emit_agg(pending.pop(0))

                while pending:
                    emit_agg(pending.pop(0))

                # ---- final coord update ----
                coordw = constp.tile([128, n_win * 3], dt.float32)
                nc.sync.dma_start(out=coordw[:], in_=coordw_d[:])
                ucm3 = constp.tile([128, n_win * 3], dt.float32)
                nc.sync.dma_start(out=ucm3[:], in_=ucm3_d[:])
                nm3 = constp.tile([128, n_win * 3], dt.float32)
                nc.sync.dma_start(out=nm3[:], in_=nm3_d[:])
                outw = constp.tile([128, n_win * 3], dt.float32)
                nc.vector.tensor_tensor(acc[:], acc[:], ucm3[:],
                                        op=mybir.AluOpType.mult)
                nc.vector.tensor_tensor(outw[:], acc[:], coordw[:],
                                        op=mybir.AluOpType.add)
                nc.vector.tensor_tensor(outw[:], outw[:], nm3[:],
                                        op=mybir.AluOpType.mult)
                nc.sync.dma_start(out=out_d[:], in_=outw[:])

    nc.compile()
    return nc


def _pad16(n):
    # 16-aligned + 128 slack entries (last gather call rounds up to x128)
    return max(-(-n // 16) * 16, 128) + 128


def kernel(**inputs):
    global N_NODES, N_EDGES, HALF
    h = np.asarray(inputs["h"], np.float32)
    N_NODES = h.shape[0]
    N_EDGES = np.asarray(inputs["edge_index"]).shape[1]
    HALF = (N_NODES + 1) // 2
    assert HALF < 32768 and N_NODES - HALF < 32768
    coord = np.asarray(inputs["coord"], np.float32)
    edge_index = np.asarray(inputs["edge_index"]).astype(np.int64)
    row, col = edge_index[0], edge_index[1]

    ins = dict(inputs)
    ins["coord"] = coord

    meta, perm, row_s, col_s, cwh = _build_schedule(row, col)
    h_bf16 = np.ascontiguousarray(h.astype(BF16))

    W1 = np.asarray(inputs["W1"], np.float32)
    W2 = np.asarray(inputs["W2"], np.float32)
    W3 = np.asarray(inputs["W3"], np.float32)
    w1c = W1[:, 2 * H].astype(BF16)                      # [H]
    shared = {
        "w1aT": np.ascontiguousarray(W1[:, :H].T.astype(BF16)),
        "w1bT": np.ascontiguousarray(W1[:, H : 2 * H].T.astype(BF16)),
        "w1crep": np.ascontiguousarray(
            np.tile(w1c, meta["n_win"]).reshape(1, -1).astype(FP8)),
        "b1": np.asarray(inputs["b1"], np.float32).reshape(H, 1),
        "w2T": np.ascontiguousarray(W2.T.astype(BF16)),
        "b2": np.asarray(inputs["b2"], np.float32).reshape(H, 1),
        "w3": np.ascontiguousarray(W3.reshape(1, H).T.astype(BF16)),
    }

    in_maps = [
        _stage_core(c, meta, ins, perm, row_s, col_s, cwh, h_bf16, shared)
        for c in range(NCORES)
    ]

    nc = _build_program(meta)
    trace = bool(os.environ.get("EU_TRACE"))
    res = run_bass_kernel_spmd(nc, in_maps, list(range(NCORES)), trace=trace)
    LAST_RUN_INFO["exec_time_ns"] = res.exec_time_ns

    n_win = meta["n_win"]
    out = np.empty((N_NODES, 3), np.float32)
    for c in range(NCORES):
        nb = meta["w0"][c] * WIN
        ne = min(meta["w1"][c] * WIN, N_NODES)
        arr = res.results[c]["out"].reshape(128, n_win, 3)[:WIN]
        arr = np.ascontiguousarray(arr.transpose(1, 0, 2)).reshape(-1, 3)
        out[nb:ne] = arr[: ne - nb]
    return out


# revision 34
# speedup vs baseline: 1.2973x; 1.0745x over previous
"""Trainium2 Bass kernel for nn_EquivariantUpdate (GNN message passing).

Strategy: sort edges by destination (row), shard across 8 NeuronCores at
window boundaries (disjoint per-core aggregates, no collective).

v4: windows of 127 destination nodes. Rows are window-local, so instead
of gathering h[row] per edge we precompute A_w = H_w @ W1a^T per window
(one matmul), place w1c in its 128th row, and expand to edges with a
host-staged one-hot whose 128th row carries edge_attr — one matmul per
tile yields W1a h_row + w1c ea. The other one-hot orientation performs
the segment-sum. Col features use dma_gather (A/B halves for int16
range), with at most one mixed A/B tile per window (two col matmuls).
PSUM is batched in 4-tile groups: one SILU per layer per group, one
wide W2 matmul, batched phi PSUM->SBUF copy, and a single broadcast
tensor_tensor for the cd*phi scaling.
"""

import os
import numpy as np
import ml_dtypes

import concourse.bacc as bacc
import concourse.mybir as mybir
import concourse.tile as tile
from concourse.bass_utils import run_bass_kernel_spmd
from concourse.library_config import mlp as mlp_lib

H = 128
NCORES = 8
WIN = 127                      # nodes per aggregation window
NORM = 100.0
N_NODES = 50000                # overwritten per-call from input shapes
N_EDGES = 400000
HALF = 25000
BF16 = ml_dtypes.bfloat16
FP8 = ml_dtypes.float8_e4m3fn
CDSCALE = 131072.0             # cd prescale so fp8 cdp avoids denormals
SC = 32                        # tiles per DMA/gather chunk
GRP = 4                        # tiles per PSUM group (512 edges)

LAST_RUN_INFO = {}             # test.py reads exec_time_ns from here

_MAXW = 1


def _patch_drain():
    import concourse.tile as tile_mod
    if getattr(tile_mod.TileContext, "_eu_drain_patched", False):
        return
    ScopedClock = tile_mod.ScopedClock

    def _drain_and_barrier(self, tick_clock, wait_clock):
        nc = self.nc
        drain_inst = nc.sync.drain()
        wait_clock.add_sem_waits(
            drain_inst.ins, ScopedClock({None: tick_clock.global_clock})
        )
        inst = drain_inst.ins
        if inst.sync_info is not None and len(inst.sync_info.on_wait) > _MAXW:
            waits = list(inst.sync_info.on_wait)
            inst.sync_info.on_wait = waits[:_MAXW]
            for k in range(_MAXW, len(waits), _MAXW):
                extra = nc.sync.drain()
                einst = extra.ins
                if einst.sync_info is None:
                    einst.sync_info = mybir.SyncInfo(
                        on_wait=waits[k : k + _MAXW], on_update=[]
                    )
                else:
                    einst.sync_info.on_wait = waits[k : k + _MAXW]
        nc.all_engine_barrier()
        popped = nc._tile_sem_poison_stack.pop()
        assert popped is self._sem_poison
        nc.clear_and_free_semaphores(list(self.sems.allocated().values()))
        nc.all_engine_barrier()

    tile_mod.TileContext._drain_and_barrier = _drain_and_barrier
    tile_mod.TileContext._eu_drain_patched = True


def _wrap_idx(a):
    """[n] int16 -> [128, n//16] wrapped in 16 partitions, replicated x8."""
    n = a.shape[0]
    w = a.reshape(n // 16, 16).T
    return np.ascontiguousarray(np.tile(w, (8, 1)))


def _build_schedule(row, col):
    """Host-side scheduling. Returns static meta + per-core data."""
    n_win_total = (N_NODES + WIN - 1) // WIN

    perm = np.argsort(row, kind="stable")
    row_s = row[perm]
    col_s = col[perm]
    gwin = row_s // WIN  # global window id per sorted edge, non-decreasing

    wcount = np.bincount(gwin, minlength=n_win_total)
    cum = np.cumsum(wcount)
    bounds = [0]
    for c in range(1, NCORES):
        target = N_EDGES * c / NCORES
        bounds.append(int(np.searchsorted(cum, target)) + 1)
    bounds.append(n_win_total)
    w0 = bounds[:-1]
    w1 = bounds[1:]
    n_win = max(b - a for a, b in zip(w0, w1))

    wstart = np.concatenate([[0], cum]).astype(np.int64)
    half_of = (col_s >= HALF)

    core_win_half = []  # [core][w] -> (idxA, idxB) arrays of sorted-edge idx
    for c in range(NCORES):
        wins = []
        for w in range(n_win):
            g = w0[c] + w
            if g < w1[c]:
                lo, hi = wstart[g], wstart[g + 1]
                sl = np.arange(lo, hi)
                m = half_of[lo:hi]
                wins.append((sl[~m], sl[m]))
            else:
                wins.append((np.empty(0, np.int64), np.empty(0, np.int64)))
        core_win_half.append(wins)

    nAmax = np.zeros(n_win, np.int64)
    nBmax = np.zeros(n_win, np.int64)
    for w in range(n_win):
        for c in range(NCORES):
            a, b = core_win_half[c][w]
            nAmax[w] = max(nAmax[w], len(a))
            nBmax[w] = max(nBmax[w], len(b))
    # A-region sizes 16-aligned so gather offsets stay 16-aligned
    nAmax = -(-nAmax // 16) * 16

    TW = -(-(nAmax + nBmax) // 128)          # tiles per window
    nBslot = TW * 128 - nAmax                # B-region slots per window
    win_first = np.concatenate([[0], np.cumsum(TW)])[:-1]
    NT = int(TW.sum())
    NS = NT * 128

    meta = dict(
        n_win=n_win, NT=NT, NS=NS,
        win_first=win_first, win_ntiles=TW,
        nAmax=nAmax, nBslot=nBslot, w0=w0, w1=w1,
        nA=int(nAmax.sum()), nB=int(nBslot.sum()),
    )
    return meta, perm, row_s, col_s, core_win_half


def _tile_info(meta):
    """Static per-tile info: (window, ka) with ka = #A-slots in tile."""
    n_win = meta["n_win"]
    TW = meta["win_ntiles"]
    nAmax = meta["nAmax"]
    info = []
    for w in range(n_win):
        for k in range(int(TW[w])):
            ka = int(min(max(nAmax[w] - k * 128, 0), 128))
            info.append((w, ka))
    return info


def _stage_core(c, meta, inputs, perm, row_s, col_s, core_win_half,
                h_bf16, shared):
    """Build the per-core input map (slot-ordered staging arrays)."""
    n_win, NT, NS = meta["n_win"], meta["NT"], meta["NS"]
    win_first = meta["win_first"]
    nAmax, nBslot = meta["nAmax"], meta["nBslot"]
    w0 = meta["w0"]
    nb = w0[c] * WIN
    rmax = n_win * WIN

    coord = inputs["coord"]
    coord_diff = inputs["coord_diff"]
    edge_attr = inputs["edge_attr"]
    edge_mask = inputs["edge_mask"]
    node_mask = inputs["node_mask"]
    ucm = inputs["update_coords_mask"]

    # slot -> sorted-edge index (or -1 for padding)
    slot_edge = np.full(NS, -1, np.int64)
    slot_win = np.zeros(NS, np.int64)
    colidxA = np.zeros(_pad16(meta["nA"]), np.int16)
    colidxB = np.zeros(_pad16(meta["nB"]), np.int16)
    aoff = 0
    boff = 0
    for w in range(n_win):
        s0 = win_first[w] * 128
        ea_, eb_ = core_win_half[c][w]
        na, nbl = int(nAmax[w]), int(nBslot[w])
        slot_win[s0 : s0 + (na + nbl)] = w
        slot_edge[s0 : s0 + len(ea_)] = ea_
        slot_edge[s0 + na : s0 + na + len(eb_)] = eb_
        colidxA[aoff : aoff + len(ea_)] = col_s[ea_]
        colidxB[boff : boff + len(eb_)] = (col_s[eb_] - HALF)
        aoff += na
        boff += nbl

    valid = slot_edge >= 0
    se = np.where(valid, slot_edge, 0)

    rowv = row_s[se]
    loc = np.where(valid, rowv - nb - slot_win * WIN, 0)

    # one-hot tiles, both orientations (bf16); ohT row 127 carries ea
    tt = np.arange(NS) // 128
    ee = np.arange(NS) % 128
    v = valid
    ohT = np.zeros((128, NT, 128), FP8)    # [j, t, e]
    ohT[loc[v], tt[v], ee[v]] = 1.0
    eav = np.where(valid, edge_attr[perm[se], 0], 0.0).astype(np.float32)
    ohT[127, tt, ee] = eav.astype(FP8)
    oh = np.zeros((128, NT, 128), FP8)     # [e, t, j]
    oh[ee[v], tt[v], loc[v]] = 1.0

    em = np.where(valid, edge_mask[perm[se], 0], 0.0).astype(np.float32)
    cd = np.where(valid[:, None],
                  coord_diff[perm[se]] * (em * (CDSCALE / NORM))[:, None],
                  0.0).astype(np.float32)

    # transposed window blocks of h rows: hrowT[f, w*127 + j]
    avail = min(rmax, N_NODES - nb)
    blk = np.zeros((rmax, H), np.float32)
    blk[:avail] = np.asarray(h_bf16[nb : nb + avail], np.float32)
    hrowT = np.ascontiguousarray(
        blk.reshape(n_win, WIN, H).transpose(2, 0, 1).reshape(H, n_win * WIN)
    ).astype(BF16)

    def swz(x, rep3=False, scale=1.0):
        d = x.shape[1] if x.ndim > 1 else 1
        flat = np.zeros((rmax, d), np.float32)
        flat[:avail] = x[nb : nb + avail].reshape(avail, d) * scale
        out = flat.reshape(n_win, WIN, d)
        if rep3 and d == 1:
            out = np.repeat(out, 3, axis=2)
        out = out.transpose(1, 0, 2).reshape(WIN, -1)
        pad = np.zeros((128, out.shape[1]), np.float32)
        pad[:WIN] = out
        return np.ascontiguousarray(pad)

    in_map = {
        "h_full": h_bf16,
        "hrowT": hrowT,
        "colidxA": _wrap_idx(colidxA),
        "colidxB": _wrap_idx(colidxB),
        "ohT": np.ascontiguousarray(ohT.reshape(128, NT * 128)),
        "oh": np.ascontiguousarray(oh.reshape(128, NT * 128)),
        "cd": np.ascontiguousarray(
            cd.reshape(NT, 128, 3).transpose(1, 0, 2).astype(BF16)),
        "coordw": swz(coord),
        "ucm3": swz(ucm, rep3=True, scale=1.0 / CDSCALE),
        "nm3": swz(node_mask, rep3=True),
    }
    in_map.update(shared)
    return in_map


def _actfn():
    if os.environ.get("EU_SIM_ACT"):
        return mybir.ActivationFunctionType.Sigmoid
    return mybir.ActivationFunctionType.Silu


def _build_program(meta):
    n_win, NT, NS = meta["n_win"], meta["NT"], meta["NS"]
    win_first, win_ntiles = meta["win_first"], meta["win_ntiles"]
    nA, nB = meta["nA"], meta["nB"]
    rmax = n_win * WIN
    tinfo = _tile_info(meta)

    _patch_drain()
    nc = bacc.Bacc("TRN2", num_swdge_queues=4)
    dt = mybir.dt
    qrr = [0]

    def nextq():
        qrr[0] = (qrr[0] + 1) % 4
        return qrr[0]

    def P(name, shape, dtype, out=False):
        return nc.declare_dram_parameter(name, shape, dtype, isOutput=out)

    h_full = P("h_full", [N_NODES, H], dt.bfloat16)
    hrowT_d = P("hrowT", [H, rmax], dt.bfloat16)
    colidxA_d = P("colidxA", [128, _pad16(nA) // 16], dt.int16)
    colidxB_d = P("colidxB", [128, _pad16(nB) // 16], dt.int16)
    ohT_d = P("ohT", [128, NT * 128], dt.float8e4)
    oh_d = P("oh", [128, NT * 128], dt.float8e4)
    cd_d = P("cd", [128, NT, 3], dt.bfloat16)
    coordw_d = P("coordw", [128, n_win * 3], dt.float32)
    ucm3_d = P("ucm3", [128, n_win * 3], dt.float32)
    nm3_d = P("nm3", [128, n_win * 3], dt.float32)
    w1aT_d = P("w1aT", [H, H], dt.bfloat16)
    w1bT_d = P("w1bT", [H, H], dt.bfloat16)
    w1crep_d = P("w1crep", [1, n_win * 128], dt.float8e4)
    b1_d = P("b1", [H, 1], dt.float32)
    w2T_d = P("w2T", [H, H], dt.bfloat16)
    b2_d = P("b2", [H, 1], dt.float32)
    w3_d = P("w3", [H, 1], dt.bfloat16)
    out_d = P("out", [128, n_win * 3], dt.float32, out=True)

    nc.gpsimd.load_library(mlp_lib)

    # per-chunk static col-slot offsets (tapered: small first/last chunks)
    sizes = []
    t = 0
    lead = [8, 16, 24]
    while t < NT:
        rem = NT - t
        if lead:
            s = lead.pop(0)
        elif rem > SC + 24:
            s = SC
        elif rem > 24:
            s = max(8, rem // 2)
        else:
            s = rem
        s = min(s, rem)
        sizes.append(s)
        t += s
    chunk_t0 = []
    t = 0
    for s in sizes:
        chunk_t0.append(t)
        t += s
    a_off = [0]
    b_off = [0]
    for ci, t0 in enumerate(chunk_t0):
        t1c = min(t0 + sizes[ci], NT)
        ca = sum(tinfo[t][1] for t in range(t0, t1c))
        cb = sum(128 - tinfo[t][1] for t in range(t0, t1c))
        a_off.append(a_off[-1] + ca)
        b_off.append(b_off[-1] + cb)

    with tile.TileContext(nc) as tc:
        with (
            tc.tile_pool(name="const", bufs=1) as constp,
            tc.tile_pool(name="gath", bufs=8) as gathp,
            tc.tile_pool(name="work", bufs=3) as workp,
        ):
            # ---- constants ----
            w1aT = constp.tile([128, H], dt.bfloat16)
            nc.sync.dma_start(out=w1aT[:], in_=w1aT_d[:])
            w1bT = constp.tile([128, H], dt.bfloat16)
            nc.sync.dma_start(out=w1bT[:], in_=w1bT_d[:])
            b1 = constp.tile([H, 1], dt.float32)
            nc.sync.dma_start(out=b1[:], in_=b1_d[:])
            w2T = constp.tile([128, H], dt.bfloat16)
            nc.sync.dma_start(out=w2T[:], in_=w2T_d[:])
            b2 = constp.tile([H, 1], dt.float32)
            nc.sync.dma_start(out=b2[:], in_=b2_d[:])
            w3 = constp.tile([H, 1], dt.bfloat16)
            nc.sync.dma_start(out=w3[:], in_=w3_d[:])
            hrowT_sb = constp.tile([128, rmax], dt.bfloat16)
            nc.sync.dma_start(out=hrowT_sb[:], in_=hrowT_d[:])
            colA_sb = constp.tile([128, _pad16(nA) // 16], dt.int16)
            nc.scalar.dma_start(out=colA_sb[:], in_=colidxA_d[:])
            colB_sb = constp.tile([128, _pad16(nB) // 16], dt.int16)
            nc.scalar.dma_start(out=colB_sb[:], in_=colidxB_d[:])
            cd_sb = constp.tile([128, NT, 3], dt.bfloat16)
            nc.sync.dma_start(out=cd_sb[:], in_=cd_d[:])

            acc = constp.tile([128, n_win * 3], dt.float32)
            nc.vector.memset(acc[:], 0.0)

            awsb = constp.tile([128, n_win * 128], dt.float8e4)
            # row 127 of every window block = w1c (pairs with ea in ohT)
            nc.sync.dma_start(out=awsb[127:128, :], in_=w1crep_d[:])

            # ---- per-window A_w = H_w @ W1a^T (rows 0..126) ----
            with tc.tile_pool(name="awps", bufs=2, space="PSUM") as awpsp:
                for w in range(n_win):
                    a_ps = awpsp.tile([128, H], dt.float32, space="PSUM",
                                      tag="aps")
                    nc.tensor.matmul(a_ps[:WIN, :],
                                     hrowT_sb[:, w * WIN : (w + 1) * WIN],
                                     w1aT[:], start=True, stop=True)
                    nc.scalar.copy(awsb[:WIN, w * 128 : w * 128 + H],
                                   a_ps[:WIN, :])

            with (
                tc.tile_pool(name="mm1ps", bufs=3, space="PSUM") as mm1ps,
                tc.tile_pool(name="mm2ps", bufs=1, space="PSUM") as mm2ps,
                tc.tile_pool(name="phips", bufs=2, space="PSUM") as phips,
                tc.tile_pool(name="aggps", bufs=2, space="PSUM") as aggps,
            ):
                agg_state = [None]
                pending = []

                def emit_agg(p):
                    g0p, ngp, t0p, ohgp, cdpgp = p
                    for i in range(ngp):
                        t = g0p + i
                        w, ka = tinfo[t]
                        first = (t == win_first[w])
                        last = (t == win_first[w] + win_ntiles[w] - 1)
                        if first:
                            agg_state[0] = aggps.tile(
                                [128, 3], dt.float32, space="PSUM",
                                tag="agg", name="agg")
                        nc.tensor.matmul(
                            agg_state[0][:],
                            ohgp[:, (t - t0p) * 128 : (t - t0p + 1) * 128],
                            cdpgp[:, i, :], start=first, stop=last)
                        if last:
                            nc.vector.tensor_copy(
                                acc[:, w * 3 : (w + 1) * 3], agg_state[0][:])

                for ci, t0 in enumerate(chunk_t0):
                    t1 = min(t0 + sizes[ci], NT)
                    nrow = (t1 - t0) * 128
                    na_c = a_off[ci + 1] - a_off[ci]
                    nb_c = b_off[ci + 1] - b_off[ci]

                    bbase = -(-na_c // 128) * 128
                    cg = gathp.tile([128, 1, SC * 128 + 256], dt.bfloat16,
                                    tag="cg")
                    GC = 2048
                    for q0 in range(0, na_c, GC):
                        qn = -(-min(GC, na_c - q0) // 128) * 128
                        nc.gpsimd.dma_gather(
                            cg[:, :, q0 : q0 + qn], h_full[0:HALF],
                            colA_sb[:, (a_off[ci] + q0) // 16 :
                                    (a_off[ci] + q0 + qn) // 16],
                            qn, qn, H, transpose=True, single_packet=False,
                            queue_num=nextq())
                    for q0 in range(0, nb_c, GC):
                        qn = -(-min(GC, nb_c - q0) // 128) * 128
                        nc.gpsimd.dma_gather(
                            cg[:, :, bbase + q0 : bbase + q0 + qn],
                            h_full[HALF:N_NODES],
                            colB_sb[:, (b_off[ci] + q0) // 16 :
                                    (b_off[ci] + q0 + qn) // 16],
                            qn, qn, H, transpose=True, single_packet=False,
                            queue_num=nextq())

                    ohTg = gathp.tile([128, SC * 128], dt.float8e4, tag="ohTg")
                    nc.sync.dma_start(
                        out=ohTg[:, :nrow],
                        in_=ohT_d[:, t0 * 128 : t0 * 128 + nrow])
                    ohg = gathp.tile([128, SC * 128], dt.float8e4, tag="ohg")
                    nc.scalar.dma_start(
                        out=ohg[:, :nrow],
                        in_=oh_d[:, t0 * 128 : t0 * 128 + nrow])

                    apos = 0
                    bpos = 0
                    for g0 in range(t0, t1, GRP):
                        g1 = min(g0 + GRP, t1)
                        ng = g1 - g0
                        nge = ng * 128

                        ps1 = mm1ps.tile([128, GRP * 128], dt.float32,
                                         space="PSUM", tag="mm1")
                        for i in range(ng):
                            t = g0 + i
                            w, ka = tinfo[t]
                            sl = slice(i * 128, (i + 1) * 128)
                            nc.tensor.matmul(
                                ps1[:, sl],
                                awsb[:, w * 128 : (w + 1) * 128],
                                ohTg[:, (t - t0) * 128 : (t - t0 + 1) * 128],
                                start=True, stop=False)
                            if ka > 0:
                                nc.tensor.matmul(
                                    ps1[:, i * 128 : i * 128 + ka], w1bT[:],
                                    cg[:, 0, apos : apos + ka],
                                    start=False, stop=True)
                                apos += ka
                            if ka < 128:
                                kb = 128 - ka
                                nc.tensor.matmul(
                                    ps1[:, i * 128 + ka : (i + 1) * 128],
                                    w1bT[:],
                                    cg[:, 0, bbase + bpos : bbase + bpos + kb],
                                    start=False, stop=True)
                                bpos += kb

                        x1 = workp.tile([128, GRP * 128], dt.bfloat16,
                                        tag="x1")
                        nc.scalar.activation(x1[:, :nge], ps1[:, :nge],
                                             _actfn(), bias=b1[:])
                        ps2 = mm2ps.tile([128, GRP * 128], dt.float32,
                                         space="PSUM", tag="mm2")
                        nc.tensor.matmul(ps2[:, :nge], w2T[:], x1[:, :nge],
                                         start=True, stop=True)
                        x2 = workp.tile([128, GRP * 128], dt.bfloat16,
                                        tag="x2")
                        nc.scalar.activation(x2[:, :nge], ps2[:, :nge],
                                             _actfn(), bias=b2[:])

                        phi_ps = phips.tile([128, GRP], dt.float32,
                                            space="PSUM", tag="phi")
                        for i in range(ng):
                            nc.tensor.matmul(phi_ps[:, i : i + 1],
                                             x2[:, i * 128 : (i + 1) * 128],
                                             w3[:], start=True, stop=True)
                        phi_sb = workp.tile([128, GRP], dt.float32,
                                            tag="phisb")
                        nc.vector.tensor_copy(phi_sb[:, :ng], phi_ps[:, :ng])
                        cdpg = workp.tile([128, GRP, 3], dt.float8e4,
                                          tag="cdpg")
                        phib = phi_sb[:, :ng].unsqueeze(2).broadcast_to(
                            [128, ng, 3])
                        nc.vector.tensor_tensor(
                            cdpg[:, :ng, :], cd_sb[:, g0 : g0 + ng, :], phib,
                            op=mybir.AluOpType.mult)

                        pending.append((g0, ng, t0, ohg, cdpg))
                        emit_agg(pending.pop(0))

                while pending:
                    emit_agg(pending.pop(0))

                # ---- final coord update ----
                coordw = constp.tile([128, n_win * 3], dt.float32)
                nc.sync.dma_start(out=coordw[:], in_=coordw_d[:])
                ucm3 = constp.tile([128, n_win * 3], dt.float32)
                nc.sync.dma_start(out=ucm3[:], in_=ucm3_d[:])
                nm3 = constp.tile([128, n_win * 3], dt.float32)
                nc.sync.dma_start(out=nm3[:], in_=nm3_d[:])
                outw = constp.tile([128, n_win * 3], dt.float32)
                nc.vector.tensor_tensor(acc[:], acc[:], ucm3[:],
                                        op=mybir.AluOpType.mult)
                nc.vector.tensor_tensor(outw[:], acc[:], coordw[:],
                                        op=mybir.AluOpType.add)
                nc.vector.tensor_tensor(outw[:], outw[:], nm3[:],
                                        op=mybir.AluOpType.mult)
                nc.sync.dma_start(out=out_d[:], in_=outw[:])

    nc.compile()
    return nc


def _pad16(n):
    # 16-aligned + 128 slack entries (last gather call rounds up to x128)
    return max(-(-n // 16) * 16, 128) + 128


def kernel(**inputs):
    global N_NODES, N_EDGES, HALF
    h = np.asarray(inputs["h"], np.float32)
    N_NODES = h.shape[0]
    N_EDGES = np.asarray(inputs["edge_index"]).shape[1]
    HALF = (N_NODES + 1) // 2
    assert HALF < 32768 and N_NODES - HALF < 32768
    coord = np.asarray(inputs["coord"], np.float32)
    edge_index = np.asarray(inputs["edge_index"]).astype(np.int64)
    row, col = edge_index[0], edge_index[1]

    ins = dict(inputs)
    ins["coord"] = coord

    meta, perm, row_s, col_s, cwh = _build_schedule(row, col)
    h_bf16 = np.ascontiguousarray(h.astype(BF16))

    W1 = np.asarray(inputs["W1"], np.float32)
    W2 = np.asarray(inputs["W2"], np.float32)
    W3 = np.asarray(inputs["W3"], np.float32)
    w1c = W1[:, 2 * H].astype(BF16)                      # [H]
    shared = {
        "w1aT": np.ascontiguousarray(W1[:, :H].T.astype(BF16)),
        "w1bT": np.ascontiguousarray(W1[:, H : 2 * H].T.astype(BF16)),
        "w1crep": np.ascontiguousarray(
            np.tile(w1c, meta["n_win"]).reshape(1, -1).astype(FP8)),
        "b1": np.asarray(inputs["b1"], np.float32).reshape(H, 1),
        "w2T": np.ascontiguousarray(W2.T.astype(BF16)),
        "b2": np.asarray(inputs["b2"], np.float32).reshape(H, 1),
        "w3": np.ascontiguousarray(W3.reshape(1, H).T.astype(BF16)),
    }

    in_maps = [
        _stage_core(c, meta, ins, perm, row_s, col_s, cwh, h_bf16, shared)
        for c in range(NCORES)
    ]

    nc = _build_program(meta)
    trace = bool(os.environ.get("EU_TRACE"))
    res = run_bass_kernel_spmd(nc, in_maps, list(range(NCORES)), trace=trace)
    LAST_RUN_INFO["exec_time_ns"] = res.exec_time_ns

    n_win = meta["n_win"]
    out = np.empty((N_NODES, 3), np.float32)
    for c in range(NCORES):
        nb = meta["w0"][c] * WIN
        ne = min(meta["w1"][c] * WIN, N_NODES)
        arr = res.results[c]["out"].reshape(128, n_win, 3)[:WIN]
        arr = np.ascontiguousarray(arr.transpose(1, 0, 2)).reshape(-1, 3)
        out[nb:ne] = arr[: ne - nb]
    return out
